# revision 4
# baseline (speedup 1.0000x reference)
"""ConvLSTM3D encoder kernel for 8 trn2 NeuronCores.

Sharding: core c in [0,8) handles batch b = c//4, z-slab k = c%4 (8 output
planes z in [8k, 8k+8)).  The sequential T=10 loop runs on-device; per-step
halo exchange (1 plane each side of the slab) goes through an AllGather over
the 4 cores of each batch group (replica groups [[0..3],[4..7]]).

Conv mapping: gates = Wx (x) x_t (stride 2) + Wh (x) h + b is computed as a
single K=128 matmul accumulation stream per 512-voxel output chunk:
  partitions  0..95  : three z-shifted copies of h (dz = 0,1,2)
  partitions 96..122 : host-precomputed im2col taps of x_t (27 taps)
  partition  123     : ones (bias row)
For each (dy,dx) in 3x3, one matmul with an AP offset of (dy,dx) into the
padded (34x34) plane layout contracts channels x dz at once; the x-conv and
bias blocks ride along in the delta=(0,0) matmul only (their lhsT rows are
zero in the other eight).

Scheduling: each step processes planes in order [0,7],[1,6] (slice A) then
[2,3,4,5] (slice B).  The halo planes (0,7) are computed first so their h
goes to the AllGather as early as possible; the collective then overlaps
slice B's matmuls and the next step's non-halo work.  The next step's first
matmul tile (planes 0,7) is the only consumer of the incoming halo.
"""

import os
import sys
from contextlib import ExitStack

import numpy as np

for _p in ("/opt/trn_rl_repo", "/root/.axon_site/_ro/trn_rl_repo"):
    if os.path.isdir(_p) and _p not in sys.path:
        sys.path.insert(0, _p)

import concourse.bass as bass
import concourse.bacc as bacc
import concourse.mybir as mybir
from concourse import tile
from concourse.bass_utils import run_bass_kernel_spmd

F32 = mybir.dt.float32
I32 = mybir.dt.int32
MM_DT = mybir.dt.float32r  # matmul operand dtype (1 cycle/row at N>=256)

T = 10
CH = 32          # hidden channels
NG = 128         # gate rows (4 gates x 32 ch)
SLAB = 8         # output planes per core
PLW = 34         # padded plane width
PL = PLW * PLW   # padded plane elements (1156)
HS_FREE = SLAB * PL  # h-stack free size per partition (9248)
DELTAS = [(dy, dx) for dy in range(3) for dx in range(3)]
# plane processing order: halo planes (0, 7) first so the AllGather of
# their h fires early; interior planes fill the collective's window.
PI = [0, 7, 1, 6, 2, 3, 4, 5]
RG = [[0, 1, 2, 3], [4, 5, 6, 7]]
NSLOT = 12       # 4 ranks x 3 agin slots per group

_prog_cache = {}


def _build_program(nsteps=T, halo=True):
    key = (nsteps, halo)
    if key in _prog_cache:
        return _prog_cache[key]

    nc = bacc.Bacc(num_devices=8)

    xim_d = nc.dram_tensor("xim", [T, 28, HS_FREE], MM_DT, kind="ExternalInput")
    whl_d = nc.dram_tensor("whl", [9, 128, 128], MM_DT, kind="ExternalInput")
    hoff_d = nc.dram_tensor("hoff", [1, 2], I32, kind="ExternalInput")
    hout_d = nc.dram_tensor("hout", [CH, SLAB, 32, 32], F32, kind="ExternalOutput")
    agin = nc.dram_tensor("agin", [3, CH, 1024], F32)
    agout = nc.dram_tensor("agout", [NSLOT, CH, 1024], F32)

    with ExitStack() as ctx:
        tc = ctx.enter_context(tile.TileContext(nc))
        pers = ctx.enter_context(tc.tile_pool(name="pers", bufs=1))
        psum = ctx.enter_context(tc.tile_pool(name="psum", bufs=2, space="PSUM"))
        work = ctx.enter_context(tc.tile_pool(name="work", bufs=2))

        hstack = [
            pers.tile([128, HS_FREE], MM_DT, tag="hstackA", name="hstackA"),
            pers.tile([128, HS_FREE], MM_DT, tag="hstackB", name="hstackB"),
        ]
        wh_sb = pers.tile([128, 9 * 128], MM_DT, tag="wh")
        gates = pers.tile([128, 16 * 512], F32, tag="gates")
        c_state = pers.tile([128, 4 * 512], F32, tag="cstate")
        zscr = pers.tile([32, 1024], F32, tag="zscr")

        # ---- init (on-chip zero fill; no HBM zero traffic) ----
        nc.vector.memset(hstack[0][:, :].bitcast(F32), 0.0)
        nc.gpsimd.memset(hstack[1][:, :].bitcast(F32), 0.0)
        nc.vector.memset(c_state[:, :], 0.0)
        nc.vector.memset(zscr[:, :], 0.0)
        nc.sync.dma_start(out=agin[2], in_=zscr[:, :])
        for _d in range(9):
            nc.sync.dma_start(out=wh_sb[:, 128 * _d:128 * (_d + 1)],
                              in_=whl_d[_d])
        nc.sync.dma_start(out=hstack[0][96:124, :], in_=xim_d[0])

        r_lo = nc.alloc_register(mybir.EngineType.Pool, "r_lo")
        r_hi = nc.alloc_register(mybir.EngineType.Pool, "r_hi")
        nc.reg_load(r_lo, hoff_d[0:1, 0:1])
        nc.reg_load(r_hi, hoff_d[0:1, 1:2])
        rv_lo = nc.snap(r_lo, min_val=0, max_val=NSLOT - 1)
        rv_hi = nc.snap(r_hi, min_val=0, max_val=NSLOT - 1)

        hsv = [h[:, :].rearrange("p (z y x) -> p z y x", z=SLAB, y=PLW, x=PLW)
               for h in hstack]

        # round-robin engine assignment for the h-stack broadcast copies
        # (vector + scalar only; gpsimd hosts the collective trigger/wait)
        bcast_engines = [nc.vector, nc.scalar]

        T_ = nsteps
        for t in range(T_):
            cur, nxt = hstack[t % 2], hstack[(t + 1) % 2]
            curv, nxtv = hsv[t % 2], hsv[(t + 1) % 2]
            last = t == T_ - 1
            if not last:
                nc.sync.dma_start(out=nxt[96:124, :], in_=xim_d[t + 1])

            eng_i = 0
            for l in range(2):  # slice A: planes (0,7),(1,6); slice B: (2,3),(4,5)
                gt = [work.tile([128, 1024], F32, tag=f"gate{G}", name=f"gate{G}")
                      for G in range(4)]
                for h in range(2):  # two psum tiles (= two planes) per slice
                    ps = psum.tile([128, 2048], F32, tag="ps", name="ps")
                    # delta-outer loop: 4 consecutive matmuls share one lhsT
                    if t == 0:
                        for cq in range(4):
                            o = 8 * l + 4 * h + cq
                            pl, hf = PI[o // 2], o % 2
                            r0 = 16 * hf
                            rhs = curv[:, pl, r0:r0 + 16, 0:32]
                            nc.tensor.matmul(ps[:, 512 * cq:512 * (cq + 1)],
                                             lhsT=wh_sb[:, 0:128],
                                             rhs=rhs, start=True, stop=True)
                    else:
                        for di, (dy, dx) in enumerate(DELTAS):
                            for cq in range(4):
                                o = 8 * l + 4 * h + cq
                                pl, hf = PI[o // 2], o % 2
                                r0 = 16 * hf
                                rhs = curv[:, pl, r0 + dy:r0 + dy + 16, dx:dx + 32]
                                nc.tensor.matmul(
                                    ps[:, 512 * cq:512 * (cq + 1)],
                                    lhsT=wh_sb[:, 128 * di:128 * (di + 1)],
                                    rhs=rhs,
                                    start=(di == 0), stop=(di == 8))
                    span = slice((8 * l + 4 * h) * 512, (8 * l + 4 * h + 4) * 512)
                    nc.scalar.activation(gates[0:96, span], ps[0:96, :],
                                         mybir.ActivationFunctionType.Sigmoid)
                    nc.scalar.activation(gates[96:128, span], ps[96:128, :],
                                         mybir.ActivationFunctionType.Tanh)
                    for G in range(4):
                        for q in (2 * h, 2 * h + 1):
                            nc.sync.dma_start(
                                out=gt[G][32 * q:32 * q + 32, :],
                                in_=gates[32 * G:32 * G + 32,
                                          (8 * l + 2 * q) * 512:
                                          (8 * l + 2 * q + 2) * 512])

                i_t, f_t, o_t, g_t = gt
                prod = work.tile([128, 1024], F32, tag="prod")
                tmp = work.tile([128, 1024], F32, tag="tmp")
                tanhc = work.tile([128, 1024], F32, tag="tanhc")
                h_t = work.tile([128, 1024], MM_DT, tag="ht")
                c_sl = c_state[:, 1024 * l:1024 * (l + 1)]

                # pointwise in halves so the first two planes' h emerges
                # as soon as their psum tile is regrouped (for l=0 the
                # first half is the halo planes 0,7 feeding the AllGather)
                for hb in range(2):
                    pp = slice(64 * hb, 64 * hb + 64)
                    nc.vector.tensor_mul(prod[pp, :], i_t[pp, :], g_t[pp, :])
                    nc.vector.tensor_mul(tmp[pp, :], f_t[pp, :], c_sl[pp, :])
                    nc.vector.tensor_add(c_sl[pp, :], prod[pp, :], tmp[pp, :])
                    nc.scalar.activation(tanhc[pp, :], c_sl[pp, :],
                                         mybir.ActivationFunctionType.Tanh)
                    nc.vector.tensor_mul(h_t[pp, :], o_t[pp, :], tanhc[pp, :])
                    if l == 0 and hb == 0 and not last:
                        # h for planes 0 and 7 -> collective input
                        nc.sync.dma_start(out=agin[0],
                                          in_=h_t[0:32, :].bitcast(F32))
                        nc.sync.dma_start(out=agin[1],
                                          in_=h_t[32:64, :].bitcast(F32))

                if l == 0 and not last and halo:
                    nc.gpsimd.collective_compute(
                        "AllGather", mybir.AluOpType.bypass, replica_groups=RG,
                        ins=[agin[:, :, :]], outs=[agout[:, :, :]])
                    halo_lo = agout[bass.ds(rv_lo, 1)].squeeze(0).rearrange(
                        "c (y x) -> c y x", y=32, x=32)
                    halo_hi = agout[bass.ds(rv_hi, 1)].squeeze(0).rearrange(
                        "c (y x) -> c y x", y=32, x=32)
                    nc.gpsimd.dma_start(out=nxtv[0:32, 0, 1:33, 1:33],
                                        in_=halo_lo.bitcast(MM_DT))
                    nc.gpsimd.dma_start(out=nxtv[64:96, 7, 1:33, 1:33],
                                        in_=halo_hi.bitcast(MM_DT))

                # distribute h to the 3 dz groups of the next h-stack
                for q in range(4):
                    pl = PI[4 * l + q]
                    src = h_t[32 * q:32 * q + 32, :]
                    src3 = src.rearrange("p (y x) -> p y x", y=32, x=32)
                    if last:
                        nc.sync.dma_start(out=hout_d[:, pl, :, :],
                                          in_=src3.bitcast(F32))
                        continue
                    for g in range(3):
                        pos = pl + 1 - g
                        if 0 <= pos <= 7:
                            eng = bcast_engines[eng_i % len(bcast_engines)]
                            eng_i += 1
                            if eng is nc.scalar:
                                eng.copy(nxtv[32 * g:32 * g + 32, pos, 1:33, 1:33],
                                         src3)
                            else:
                                eng.tensor_copy(
                                    nxtv[32 * g:32 * g + 32, pos, 1:33, 1:33],
                                    src3)

    nc.finalize()
    _prog_cache[key] = nc
    return nc


def _host_inputs(input_batch, Wx, Wh, b):
    input_batch = np.asarray(input_batch, dtype=np.float32)
    Wx = np.asarray(Wx, dtype=np.float32)
    Wh = np.asarray(Wh, dtype=np.float32)
    b = np.asarray(b, dtype=np.float32)

    xp = np.zeros((2, T, 66, 66, 66), np.float32)
    xp[:, :, 1:65, 1:65, 1:65] = input_batch[:, :, 0]

    whl = np.zeros((9, 128, 128), np.float32)
    for di, (dy, dx) in enumerate(DELTAS):
        for g in range(3):
            whl[di, 32 * g:32 * g + 32, :] = Wh[:, :, g, dy, dx].T
    whl[0, 96:123, :] = Wx[:, 0].reshape(128, 27).T
    whl[0, 123, :] = b

    in_maps = []
    for c in range(8):
        bidx, k = divmod(c, 4)
        z0 = 8 * k
        xim = np.zeros((T, 28, SLAB, PLW, PLW), np.float32)
        for tz in range(3):
            for ty in range(3):
                for tx in range(3):
                    tap = tz * 9 + ty * 3 + tx
                    xim[:, tap, :, 0:32, 0:32] = xp[
                        bidx, :, 2 * z0 + tz:2 * z0 + tz + 16:2,
                        ty:ty + 64:2, tx:tx + 64:2]
        xim[:, 27, :, 0:32, 0:32] = 1.0
        # group-local rank r = k; agout has 3 slots per rank
        lo_slot = k * 3 + 2 if k == 0 else (k - 1) * 3 + 1
        hi_slot = k * 3 + 2 if k == 3 else (k + 1) * 3 + 0
        in_maps.append({
            "xim": xim.reshape(T, 28, HS_FREE),
            "whl": whl,
            "hoff": np.array([[lo_slot, hi_slot]], np.int32),
        })
    return in_maps


def run_cores(in_maps, nsteps=T, halo=True, **kwargs):
    nc = _build_program(nsteps, halo)
    return run_bass_kernel_spmd(nc, in_maps, list(range(8)), **kwargs)


def kernel(input_batch, Wx, Wh, b):
    in_maps = _host_inputs(input_batch, Wx, Wh, b)
    res = run_cores(in_maps)
    out = np.zeros((2, CH, 32, 32, 32), np.float32)
    for c in range(8):
        bidx, k = divmod(c, 4)
        out[bidx, :, 8 * k:8 * k + 8] = res.results[c]["hout"]
    return out


# revision 7
# speedup vs baseline: 1.2282x; 1.2282x over previous
"""ConvLSTM3D encoder kernel for 8 trn2 NeuronCores.

Sharding: core c in [0,8) handles batch b = c//4, z-slab k = c%4 (8 output
planes z in [8k, 8k+8)).  The sequential T=10 loop runs on-device; per-step
halo exchange (1 plane each side of the slab) goes through an AllGather over
all 8 cores (shared-output collective).

Conv mapping: gates = Wx (x) x_t (stride 2) + Wh (x) h + b is computed as a
single K=128 matmul accumulation stream per 512-voxel output chunk:
  partitions  0..95  : three z-shifted copies of h (dz = 0,1,2)
  partitions 96..122 : host-precomputed im2col taps of x_t (27 taps)
  partition  123     : ones (bias row)
For each (dy,dx) in 3x3, one matmul with an AP offset of (dy,dx) into the
padded (34x34) plane layout contracts channels x dz at once; the x-conv and
bias blocks ride along in the delta=(0,0) matmul only (their lhsT rows are
zero in the other eight).

Scheduling: 4 psum tiles per step, each covering one plane pair, processed
in order (2,3),(4,5),(1,6),(0,7).  The halo-dependent pair (0,7) is last so
the previous step's AllGather has the three interior tiles (~30us of
matmuls) to complete; its pointwise output feeds this step's AllGather,
which fires ~5us after the last matmul.  Pointwise+broadcast for tile x is
emitted while tile x+1's matmuls stream, so the LSTM update rides under the
matmul phase and the next step's first tile has its inputs ready the moment
the tensor engine frees up.
"""

import os
import sys
from contextlib import ExitStack

import numpy as np

for _p in ("/opt/trn_rl_repo", "/root/.axon_site/_ro/trn_rl_repo"):
    if os.path.isdir(_p) and _p not in sys.path:
        sys.path.insert(0, _p)

import concourse.bass as bass
import concourse.bacc as bacc
import concourse.mybir as mybir
from concourse import tile
from concourse.bass_utils import run_bass_kernel_spmd

F32 = mybir.dt.float32
I32 = mybir.dt.int32
MM_DT = mybir.dt.float32r  # matmul operand dtype (1 cycle/row at N>=256)

T = 10
CH = 32          # hidden channels
SLAB = 8         # output planes per core
PLW = 34         # padded plane width
PL = PLW * PLW   # padded plane elements (1156)
HS_FREE = SLAB * PL  # h-stack free size per partition (9248)
DELTAS = [(dy, dx) for dy in range(3) for dx in range(3)]
# plane pairs per psum tile, in processing order: interior first, the
# halo-dependent pair (0,7) last
PAIRS = [(2, 3), (4, 5), (1, 6), (0, 7)]
RG = [[0, 1, 2, 3, 4, 5, 6, 7]]

_prog_cache = {}


def _build_program(nsteps=T, halo=True):
    key = (nsteps, halo)
    if key in _prog_cache:
        return _prog_cache[key]

    nc = bacc.Bacc(num_devices=8)

    xim_d = nc.dram_tensor("xim", [T, 28, HS_FREE], MM_DT, kind="ExternalInput")
    whl_d = nc.dram_tensor("whl", [9, 128, 128], MM_DT, kind="ExternalInput")
    hoff_d = nc.dram_tensor("hoff", [1, 2], I32, kind="ExternalInput")
    hout_d = nc.dram_tensor("hout", [CH, SLAB, 32, 32], F32, kind="ExternalOutput")
    agin = nc.dram_tensor("agin", [3, CH, 1024], F32)
    agout = nc.dram_tensor("agout", [24, CH, 1024], F32, addr_space="Shared")

    with ExitStack() as ctx:
        tc = ctx.enter_context(tile.TileContext(nc))
        pers = ctx.enter_context(tc.tile_pool(name="pers", bufs=1))
        psum = ctx.enter_context(tc.tile_pool(name="psum", bufs=2, space="PSUM"))
        work = ctx.enter_context(tc.tile_pool(name="work", bufs=2))

        hstack = [
            pers.tile([128, HS_FREE], MM_DT, tag="hstackA", name="hstackA"),
            pers.tile([128, HS_FREE], MM_DT, tag="hstackB", name="hstackB"),
        ]
        wh_sb = pers.tile([128, 9 * 128], MM_DT, tag="wh")
        gates = pers.tile([128, 16 * 512], F32, tag="gates")
        c_state = pers.tile([64, 4096], F32, tag="cstate")
        zscr = pers.tile([32, 1024], F32, tag="zscr")

        # ---- init (on-chip zero fill; no HBM zero traffic) ----
        nc.vector.memset(hstack[0][:, :].bitcast(F32), 0.0)
        nc.vector.memset(hstack[1][:, :].bitcast(F32), 0.0)
        nc.vector.memset(c_state[:, :], 0.0)
        nc.vector.memset(zscr[:, :], 0.0)
        nc.sync.dma_start(out=agin[2], in_=zscr[:, :])
        for _d in range(9):
            nc.sync.dma_start(out=wh_sb[:, 128 * _d:128 * (_d + 1)],
                              in_=whl_d[_d])
        nc.sync.dma_start(out=hstack[0][96:124, :], in_=xim_d[0])

        r_lo = nc.alloc_register(mybir.EngineType.Pool, "r_lo")
        r_hi = nc.alloc_register(mybir.EngineType.Pool, "r_hi")
        nc.reg_load(r_lo, hoff_d[0:1, 0:1])
        nc.reg_load(r_hi, hoff_d[0:1, 1:2])
        rv_lo = nc.snap(r_lo, min_val=0, max_val=23)
        rv_hi = nc.snap(r_hi, min_val=0, max_val=23)

        hsv = [h[:, :].rearrange("p (z y x) -> p z y x", z=SLAB, y=PLW, x=PLW)
               for h in hstack]

        # round-robin engine assignment for the h-stack broadcast copies
        # (vector + scalar only; gpsimd hosts the collective trigger/wait)
        bcast_engines = [nc.vector, nc.scalar]
        eng_state = [0]

        def emit_tile_mm(x, curv, t):
            """matmuls + activation + gate regroup for plane pair x"""
            ps = psum.tile([128, 2048], F32, tag="ps", name="ps")
            if t == 0:
                for cq in range(4):
                    pl, hf = PAIRS[x][cq // 2], cq % 2
                    r0 = 16 * hf
                    rhs = curv[:, pl, r0:r0 + 16, 0:32]
                    nc.tensor.matmul(ps[:, 512 * cq:512 * (cq + 1)],
                                     lhsT=wh_sb[:, 0:128],
                                     rhs=rhs, start=True, stop=True)
            else:
                for di, (dy, dx) in enumerate(DELTAS):
                    for cq in range(4):
                        pl, hf = PAIRS[x][cq // 2], cq % 2
                        r0 = 16 * hf
                        rhs = curv[:, pl, r0 + dy:r0 + dy + 16, dx:dx + 32]
                        nc.tensor.matmul(
                            ps[:, 512 * cq:512 * (cq + 1)],
                            lhsT=wh_sb[:, 128 * di:128 * (di + 1)],
                            rhs=rhs,
                            start=(di == 0), stop=(di == 8))
            span = slice(4 * x * 512, (4 * x + 4) * 512)
            nc.scalar.activation(gates[0:96, span], ps[0:96, :],
                                 mybir.ActivationFunctionType.Sigmoid)
            nc.scalar.activation(gates[96:128, span], ps[96:128, :],
                                 mybir.ActivationFunctionType.Tanh)
            gt = [work.tile([64, 1024], F32, tag=f"gate{G}", name=f"gate{G}")
                  for G in range(4)]
            for G in range(4):
                for q in range(2):
                    nc.sync.dma_start(
                        out=gt[G][32 * q:32 * q + 32, :],
                        in_=gates[32 * G:32 * G + 32,
                                  (4 * x + 2 * q) * 512:
                                  (4 * x + 2 * q + 2) * 512])
            return gt

        def emit_tile_pw(x, gt, nxtv, t):
            """LSTM pointwise + h distribution for plane pair x"""
            last = t == nsteps - 1
            i_t, f_t, o_t, g_t = gt
            prod = work.tile([64, 1024], F32, tag="prod")
            tmp = work.tile([64, 1024], F32, tag="tmp")
            tanhc = work.tile([64, 1024], F32, tag="tanhc")
            h_t = work.tile([64, 1024], MM_DT, tag="ht")
            c_sl = c_state[:, 1024 * x:1024 * x + 1024]
            nc.vector.tensor_mul(prod[:, :], i_t[:, :], g_t[:, :])
            nc.vector.tensor_mul(tmp[:, :], f_t[:, :], c_sl)
            nc.vector.tensor_add(c_sl, prod[:, :], tmp[:, :])
            nc.scalar.activation(tanhc[:, :], c_sl,
                                 mybir.ActivationFunctionType.Tanh)
            nc.vector.tensor_mul(h_t[:, :], o_t[:, :], tanhc[:, :])

            if x == 3 and not last:
                # h for planes 0 and 7 -> collective input, then fire the
                # AllGather; its wire time hides under the next step's
                # interior-tile matmuls
                nc.sync.dma_start(out=agin[0], in_=h_t[0:32, :].bitcast(F32))
                nc.sync.dma_start(out=agin[1], in_=h_t[32:64, :].bitcast(F32))
                if halo:
                    nc.gpsimd.collective_compute(
                        "AllGather", mybir.AluOpType.bypass, replica_groups=RG,
                        ins=[agin[:, :, :]], outs=[agout[:, :, :]])
                    halo_lo = agout[bass.ds(rv_lo, 1)].squeeze(0).rearrange(
                        "c (y x) -> c y x", y=32, x=32)
                    halo_hi = agout[bass.ds(rv_hi, 1)].squeeze(0).rearrange(
                        "c (y x) -> c y x", y=32, x=32)
                    nc.gpsimd.dma_start(out=nxtv[0:32, 0, 1:33, 1:33],
                                        in_=halo_lo.bitcast(MM_DT))
                    nc.gpsimd.dma_start(out=nxtv[64:96, 7, 1:33, 1:33],
                                        in_=halo_hi.bitcast(MM_DT))

            for q in range(2):
                pl = PAIRS[x][q]
                src = h_t[32 * q:32 * q + 32, :]
                src3 = src.rearrange("p (y x) -> p y x", y=32, x=32)
                if last:
                    nc.sync.dma_start(out=hout_d[:, pl, :, :],
                                      in_=src3.bitcast(F32))
                    continue
                for g in range(3):
                    pos = pl + 1 - g
                    if 0 <= pos <= 7:
                        eng = bcast_engines[eng_state[0] % len(bcast_engines)]
                        eng_state[0] += 1
                        if eng is nc.scalar:
                            eng.copy(nxtv[32 * g:32 * g + 32, pos, 1:33, 1:33],
                                     src3)
                        else:
                            eng.tensor_copy(
                                nxtv[32 * g:32 * g + 32, pos, 1:33, 1:33],
                                src3)

        T_ = nsteps
        for t in range(T_):
            curv, nxtv = hsv[t % 2], hsv[(t + 1) % 2]
            nxt = hstack[(t + 1) % 2]
            if t + 1 < T_:
                nc.sync.dma_start(out=nxt[96:124, :], in_=xim_d[t + 1])

            gts = [None] * 4
            for x in range(4):
                if x >= 1:
                    emit_tile_pw(x - 1, gts[x - 1], nxtv, t)
                gts[x] = emit_tile_mm(x, curv, t)
            emit_tile_pw(3, gts[3], nxtv, t)

    nc.finalize()
    _prog_cache[key] = nc
    return nc


def _host_inputs(input_batch, Wx, Wh, b):
    input_batch = np.asarray(input_batch, dtype=np.float32)
    Wx = np.asarray(Wx, dtype=np.float32)
    Wh = np.asarray(Wh, dtype=np.float32)
    b = np.asarray(b, dtype=np.float32)

    xp = np.zeros((2, T, 66, 66, 66), np.float32)
    xp[:, :, 1:65, 1:65, 1:65] = input_batch[:, :, 0]

    whl = np.zeros((9, 128, 128), np.float32)
    for di, (dy, dx) in enumerate(DELTAS):
        for g in range(3):
            whl[di, 32 * g:32 * g + 32, :] = Wh[:, :, g, dy, dx].T
    whl[0, 96:123, :] = Wx[:, 0].reshape(128, 27).T
    whl[0, 123, :] = b

    in_maps = []
    for c in range(8):
        bidx, k = divmod(c, 4)
        z0 = 8 * k
        xim = np.zeros((T, 28, SLAB, PLW, PLW), np.float32)
        for tz in range(3):
            for ty in range(3):
                for tx in range(3):
                    tap = tz * 9 + ty * 3 + tx
                    xim[:, tap, :, 0:32, 0:32] = xp[
                        bidx, :, 2 * z0 + tz:2 * z0 + tz + 16:2,
                        ty:ty + 64:2, tx:tx + 64:2]
        xim[:, 27, :, 0:32, 0:32] = 1.0
        lo_slot = c * 3 + 2 if k == 0 else (c - 1) * 3 + 1
        hi_slot = c * 3 + 2 if k == 3 else (c + 1) * 3 + 0
        in_maps.append({
            "xim": xim.reshape(T, 28, HS_FREE),
            "whl": whl,
            "hoff": np.array([[lo_slot, hi_slot]], np.int32),
        })
    return in_maps


def run_cores(in_maps, nsteps=T, halo=True, **kwargs):
    nc = _build_program(nsteps, halo)
    return run_bass_kernel_spmd(nc, in_maps, list(range(8)), **kwargs)


def kernel(input_batch, Wx, Wh, b):
    in_maps = _host_inputs(input_batch, Wx, Wh, b)
    res = run_cores(in_maps)
    out = np.zeros((2, CH, 32, 32, 32), np.float32)
    for c in range(8):
        bidx, k = divmod(c, 4)
        out[bidx, :, 8 * k:8 * k + 8] = res.results[c]["hout"]
    return out
